# revision 4
# baseline (speedup 1.0000x reference)
"""Trainium2 Bass kernel: AnchorEncoder (cosine-sim argmax anchor retrieval + linear).

Math (per row f of features):
    idx  = argmax_c  (f . a_c) / max(||a_c||, eps)      (||f|| factor is argmax-invariant)
    out  = anchors[idx] @ W1 + f @ W2                   (W1 = W_out[:H], W2 = W_out[H:])

Distribution: data-parallel over 8 NeuronCores, 4096 feature rows per core;
anchors and W_out replicated. Per core:
  - sim matmul in bf16:  psum[128m, 1024c] += fT_chunk.T @ aTn_chunk   (aTn = normalized anchors^T)
  - argmax via VectorE max/max_index on the sim row
  - AW1 = anchors @ W1 precomputed once on-device (bf16), stored to DRAM scratch
  - per m-tile: indirect-DMA gather AW1[idx] and add to the f @ W2 psum

Host side only reshapes/shards: features and anchors are passed transposed
(H-major) and anchors zero-padded C 1000->1024; all arithmetic is on-device.
"""

import sys
import types
from contextlib import ExitStack

import numpy as np

import concourse.bass as bass
import concourse.tile as tile
from concourse import bacc, mybir

P = 128
H = 1024          # feature dim
C_RAW = 1000      # anchors
C = 1024          # padded anchors
OUT = 1024        # output dim
N_FULL = 32768    # total rows
N_CORES = 8
EPS = 1e-8

F32 = mybir.dt.float32
BF16 = mybir.dt.bfloat16
U32 = mybir.dt.uint32

HC = H // P       # 8 h-chunks
CT = C // P       # 8 anchor tiles
WC = 2 * H // P   # 16 W chunks


def _build_program(m_rows: int):
    """Build + compile the per-core Bass program for an m_rows shard."""
    mt_tiles = m_rows // P
    nc = bacc.Bacc("TRN2", target_bir_lowering=False, debug=False,
                   num_devices=N_CORES)

    ft = nc.dram_tensor("ft", [H, m_rows], F32, kind="ExternalInput").ap()
    at = nc.dram_tensor("at", [H, C], F32, kind="ExternalInput").ap()
    w = nc.dram_tensor("w", [2 * H, OUT], F32, kind="ExternalInput").ap()
    out = nc.dram_tensor("out", [m_rows, OUT], F32, kind="ExternalOutput").ap()

    ft_r = ft.rearrange("(o p) m -> o p m", p=P)
    at_r = at.rearrange("(o p) c -> o p c", p=P)
    w_r = w.rearrange("(o p) n -> o p n", p=P)
    out_r = out.rearrange("(o p) n -> o p n", p=P)

    with tile.TileContext(nc) as tc, ExitStack() as ctx:
        res_pool = ctx.enter_context(tc.tile_pool(name="resident", bufs=1))
        ps_pool = ctx.enter_context(tc.tile_pool(name="ps", bufs=8, space="PSUM"))
        dram_pool = ctx.enter_context(tc.tile_pool(name="dram", bufs=1, space="DRAM"))

        aw1 = dram_pool.tile([C, OUT], BF16, tag="aw1")
        aw1_r = aw1[:].rearrange("(o p) n -> o p n", p=P)

        ones = res_pool.tile([P, P], BF16, tag="ones")
        nc.vector.memset(ones[:], 1.0)

        # resident packed tiles (avoid 4KB-per-tile padding waste)
        wb_all = res_pool.tile([P, WC * OUT], BF16, tag="wb")    # 32KB/part
        atn_all = res_pool.tile([P, HC * C], BF16, tag="atn")    # 16KB/part
        ftb_all = res_pool.tile([P, HC * m_rows], BF16, tag="ftb")

        def wb(i):
            return wb_all[:, i * OUT:(i + 1) * OUT]

        def atn(hc):
            return atn_all[:, hc * C:(hc + 1) * C]

        def ftb(hc):
            return ftb_all[:, hc * m_rows:(hc + 1) * m_rows]

        # ---- epoch 1: anchors/W prep + AW1 (pools close before fT streaming)
        with tc.tile_pool(name="phase0", bufs=1) as p0, \
             tc.tile_pool(name="stage", bufs=2) as stg, \
             tc.tile_pool(name="awsb", bufs=2) as awp:

            # ---- W_out load + cast to bf16
            for wc in range(WC):
                s = stg.tile([P, OUT], F32, tag="wstag")
                nc.sync.dma_start(s[:], w_r[wc])
                nc.vector.tensor_copy(wb(wc), s[:])

            # ---- anchors^T load + cast to bf16 (padded)
            atb_all = p0.tile([P, HC * C], BF16, tag="atb")      # 16KB/part

            def atb(hc):
                return atb_all[:, hc * C:(hc + 1) * C]

            for hc in range(HC):
                s = stg.tile([P, C], F32, tag="wstag")
                nc.sync.dma_start(s[:], at_r[hc])
                nc.vector.tensor_copy(atb(hc), s[:])

            # ---- per-anchor 1/max(||a||,eps), broadcast on all partitions:
            # ssq[p, c] = sum_h aT[h, c]^2 via all-ones matmul (bf16 squares)
            ps_q0 = ps_pool.tile([P, 512], F32, space="PSUM", tag="ps")
            ps_q1 = ps_pool.tile([P, 512], F32, space="PSUM", tag="ps")
            for hc in range(HC):
                sq = stg.tile([P, C], BF16, tag="wstag")
                nc.vector.tensor_mul(sq[:], atb(hc), atb(hc))
                nc.tensor.matmul(ps_q0[:], ones[:], sq[:, 0:512],
                                 start=(hc == 0), stop=(hc == HC - 1))
                nc.tensor.matmul(ps_q1[:], ones[:], sq[:, 512:1024],
                                 start=(hc == 0), stop=(hc == HC - 1))
            ssq = p0.tile([P, C], F32, tag="ssq")
            nc.vector.tensor_copy(ssq[:, 0:512], ps_q0[:])
            nc.vector.tensor_copy(ssq[:, 512:1024], ps_q1[:])
            nc.vector.tensor_scalar_max(ssq[:], ssq[:], EPS * EPS)
            nrm = p0.tile([P, C], F32, tag="nrm")
            nc.scalar.sqrt(nrm[:], ssq[:])
            rsc = ssq  # ssq is dead; reuse its slot for the reciprocal
            nc.vector.reciprocal(rsc[:], nrm[:])

            # ---- normalized anchor^T in bf16
            for hc in range(HC):
                nc.vector.tensor_mul(atn(hc), atb(hc), rsc[:])

            # ---- AW1 = anchors @ W1  (bf16), written to DRAM scratch
            for ct in range(CT):
                pa0 = ps_pool.tile([P, 512], F32, space="PSUM", tag="ps")
                pa1 = ps_pool.tile([P, 512], F32, space="PSUM", tag="ps")
                for hc in range(HC):
                    lhsT = atb_all[:, hc * C + ct * P:hc * C + (ct + 1) * P]
                    nc.tensor.matmul(pa0[:], lhsT, wb(hc)[:, 0:512],
                                     start=(hc == 0), stop=(hc == HC - 1))
                    nc.tensor.matmul(pa1[:], lhsT, wb(hc)[:, 512:1024],
                                     start=(hc == 0), stop=(hc == HC - 1))
                sb = awp.tile([P, OUT], BF16, tag="awsb")
                nc.scalar.copy(sb[:, 0:512], pa0[:])
                nc.scalar.copy(sb[:, 512:1024], pa1[:])
                nc.sync.dma_start(aw1_r[ct], sb[:])

        # ---- epoch 2: stream features^T (cast to bf16) + main loop
        mt_pool = ctx.enter_context(tc.tile_pool(name="mt", bufs=3))
        ftstg_pool = ctx.enter_context(tc.tile_pool(name="ftstg", bufs=2))
        HALF = m_rows // 2
        for hc in range(HC):
            for half in range(2):
                s = ftstg_pool.tile([P, HALF], F32, tag="ftstag")
                nc.sync.dma_start(s[:], ft_r[hc, :, half * HALF:(half + 1) * HALF])
                nc.vector.tensor_copy(
                    ftb(hc)[:, half * HALF:(half + 1) * HALF], s[:])

        # ---- main loop over 128-row m-tiles
        for mt in range(mt_tiles):
            ps_s0 = ps_pool.tile([P, 512], F32, space="PSUM", tag="ps")
            ps_s1 = ps_pool.tile([P, 512], F32, space="PSUM", tag="ps")
            ps_o0 = ps_pool.tile([P, 512], F32, space="PSUM", tag="ps")
            ps_o1 = ps_pool.tile([P, 512], F32, space="PSUM", tag="ps")
            for hc in range(HC):
                lhsT = ftb(hc)[:, mt * P:(mt + 1) * P]
                first, last = hc == 0, hc == HC - 1
                nc.tensor.matmul(ps_s0[:], lhsT, atn(hc)[:, 0:512],
                                 start=first, stop=last)
                nc.tensor.matmul(ps_s1[:], lhsT, atn(hc)[:, 512:1024],
                                 start=first, stop=last)
                nc.tensor.matmul(ps_o0[:], lhsT, wb(HC + hc)[:, 0:512],
                                 start=first, stop=last)
                nc.tensor.matmul(ps_o1[:], lhsT, wb(HC + hc)[:, 512:1024],
                                 start=first, stop=last)

            sim = mt_pool.tile([P, C], F32, tag="sim")
            nc.scalar.copy(sim[:, 0:512], ps_s0[:])
            nc.scalar.copy(sim[:, 512:1024], ps_s1[:])
            mxmi = mt_pool.tile([P, 16], F32, tag="mxmi")
            mx = mxmi[:, 0:8]
            mi = mxmi[:, 8:16].bitcast(U32)
            nc.vector.max(mx, sim[:])
            nc.vector.max_index(mi, mx, sim[:])

            gath = mt_pool.tile([P, OUT], BF16, tag="gath")
            nc.gpsimd.indirect_dma_start(
                out=gath[:],
                out_offset=None,
                in_=aw1[:],
                in_offset=bass.IndirectOffsetOnAxis(ap=mxmi[:, 8:9].bitcast(U32),
                                                    axis=0),
            )

            osb = mt_pool.tile([P, OUT], F32, tag="osb")
            nc.vector.tensor_add(osb[:, 0:512], ps_o0[:], gath[:, 0:512])
            nc.vector.tensor_add(osb[:, 512:1024], ps_o1[:], gath[:, 512:1024])
            nc.sync.dma_start(out_r[mt], osb[:])

    nc.compile()
    return nc


_PROGRAM_CACHE: dict[int, object] = {}


def _get_program(m_rows: int):
    if m_rows not in _PROGRAM_CACHE:
        _PROGRAM_CACHE[m_rows] = _build_program(m_rows)
    return _PROGRAM_CACHE[m_rows]


def _prep_in_maps(features, class_anchors, W_out):
    features = np.ascontiguousarray(np.asarray(features, dtype=np.float32))
    class_anchors = np.asarray(class_anchors, dtype=np.float32)
    W_out = np.ascontiguousarray(np.asarray(W_out, dtype=np.float32))

    at = np.zeros((H, C), dtype=np.float32)
    at[:, :C_RAW] = class_anchors.T
    at = np.ascontiguousarray(at)

    in_maps = []
    n = features.shape[0]
    m = n // N_CORES
    for i in range(N_CORES):
        ft_shard = np.ascontiguousarray(features[i * m:(i + 1) * m].T)
        in_maps.append({"ft": ft_shard, "at": at, "w": W_out})
    return in_maps, m


def _install_ntff_shim():
    """This image's `antenv` lacks `axon_hooks`; provide it and install the
    ctypes NTFF profiling hook so run_bass_kernel_spmd(trace=True) works."""
    if "antenv.axon_hooks" in sys.modules:
        return
    m = types.ModuleType("antenv.axon_hooks")
    m._hook = None
    m.set_axon_ntff_profile_hook = lambda h: setattr(m, "_hook", h)
    m.get_axon_ntff_profile_hook = lambda: m._hook
    sys.modules["antenv.axon_hooks"] = m
    try:
        if "/root/.axon_site" not in sys.path:
            sys.path.insert(0, "/root/.axon_site")
        from trn_agent_boot.trn_boot import _ntff_profile_via_ctypes
        m.set_axon_ntff_profile_hook(
            _ntff_profile_via_ctypes("/opt/axon/libaxon_pjrt.so"))
    except Exception:
        pass
    import concourse.bass_utils as bass_utils
    bass_utils.upload_artifacts = lambda tmpdir: f"local:{tmpdir}"


LAST_RESULT = None


def run(features, class_anchors, W_out, trace=False):
    """Run the distributed kernel; returns (full_output, exec_time_ns|None)."""
    global LAST_RESULT
    from concourse.bass_utils import run_bass_kernel_spmd
    if trace:
        _install_ntff_shim()
    in_maps, m = _prep_in_maps(features, class_anchors, W_out)
    nc = _get_program(m)
    res = run_bass_kernel_spmd(nc, in_maps, core_ids=list(range(N_CORES)),
                               trace=trace)
    LAST_RESULT = res
    full = np.concatenate([res.results[i]["out"] for i in range(N_CORES)], axis=0)
    return full, res.exec_time_ns


def kernel(features, class_anchors, W_out):
    out, _ = run(features, class_anchors, W_out, trace=False)
    return out
